# revision 20
# baseline (speedup 1.0000x reference)
"""Trainium2 Bass kernel for GQA attention prefill (nn_Attention_60593398612481).

Full-input contract: kernel(**inputs) takes the unsharded inputs and returns
the full [B, S, DIM] fp32 output. Internally: tensor-parallel across heads on
8 NeuronCores (q-heads 4c..4c+3 + kv-head c on core c; wo row-sharded), each
core computes a full-shape partial of the output projection, host sums the 8
partials (row-parallel "all-reduce" realized at gather time).

Assumes the mask input is the standard causal mask (0 on/below diagonal,
-1e9 above) as produced by the reference setup_inputs().

Schedule (single instruction stream, engines overlap via Tile deps):
- cb 0 runs kt-major in two passes (m0-3, then k + v re-reading x from SBUF)
  so PE keeps pace with the fine-chunked startup DMA stream (no single wait
  crosses the ~3us clock-rewarm threshold); from cb 1 onward, attention
  chunks are interleaved between projection m-chains so their exp/softmax
  chains hide under projection matmuls:
    cb1+attn(b0,j0) ... cb4+(b0,j3), cb5+(b1,j0) ... cb7+(b1,j2)
- Tail: attention (b1,j3) interleaved with the output projection blocks;
  once attention is done its PSUM banks join the projection rotation.
- Attention is emitted j-outer / h-inner; scores+exp run one kb-PAIR ahead
  of the ctx matmuls (one [128,2,512] exp per off-diagonal pair keeps Act
  ahead of PE), chain tails fold into the next chain's head, and the fully
  masked columns of diagonal blocks are skipped in scores/exp.
- Causal masking of diagonal 128-blocks is folded into the scores PSUM
  accumulation as one extra PE matmul (ident.T @ tril(-1000)) so exp of the
  masked region underflows to 0 -- no Pool/DVE op sits between exp and the
  ctx matmul (the old Pool triangle-mask was the dominant PE stall).
- x/wqkv/wo are host-blocked so every DMA reads long contiguous
  per-partition lines (x 16KB, w >=1KB); x streams in quarter-cb tiles
  (6-slot ring) so prefetch slots release 4x finer than the consuming
  chain; x comes in pre-transposed, weights column-sharded, so qT/kT leave
  the QKV matmul with head-dim on partitions; RoPE (even|odd permuted) is
  fused into the PSUM eviction on DVE (evict-first so the PSUM bank frees
  after one op); v is projected directly into natural [kpos, hd] layout by
  swapping the matmul operand roles (x-tile stationary, wv moving), so no
  transposes or Act copies sit on the v path.
- Softmax runs without max-subtraction (|s| <~ 10); denominator = ones-matmul
  partition sum; reciprocal via the fast custom-DVE approx (~5x faster than
  the iterative-divide reciprocal, well within tolerance).
- Engine split: exp on Act, dn-accumulate + rope + normalize + v-evict on
  DVE, recip-broadcast on Pool, psum evictions DVE while attention is live,
  Act afterwards.
"""

import math
from dataclasses import dataclass

import numpy as np
import ml_dtypes

import concourse.bass as bass
import concourse.mybir as mybir
import concourse.tile as tile
from concourse import bacc
from concourse.masks import make_identity

BF16 = mybir.dt.bfloat16
F32 = mybir.dt.float32
AF = mybir.ActivationFunctionType


@dataclass(frozen=True)
class Cfg:
    B: int = 2
    S: int = 2048
    DIM: int = 4096
    NQ: int = 4        # q heads per core
    HD: int = 128
    CB: int = 512      # phase-1 column block (rows of x)
    QBLK: int = 512    # attention q block (PSUM bank)
    KBLK: int = 128    # attention k block (partition dim)
    NBLK: int = 512    # phase-3 out-dim block

    @property
    def R(self):
        return self.B * self.S

    @property
    def KT(self):
        return self.DIM // 128

    @property
    def NM(self):
        return self.NQ + 2  # q heads + k + v


def build_nc(cfg: Cfg, reps: int = 1):
    """Build the single-core Bass program (SPMD: same program on 8 cores)."""
    nc = bacc.Bacc("TRN2", target_bir_lowering=False)
    B, S, DIM, NQ = cfg.B, cfg.S, cfg.DIM, cfg.NQ
    R, KT, NM = cfg.R, cfg.KT, cfg.NM
    CB, QBLK, KBLK = cfg.CB, cfg.QBLK, cfg.KBLK
    NBLK = cfg.NBLK
    NCB = R // CB
    ST = S // 128          # seq row-tiles per batch
    DIAG = QBLK // KBLK    # diagonal k-blocks per q-block
    NN = DIM // NBLK
    KTH = KT // 2

    # host-blocked layouts: every DMA reads long contiguous per-partition
    # lines (x: 16KB, w: >=1KB) instead of 1KB/256B strided runs
    xb = nc.dram_tensor("xb", [NCB, 128, KT, CB], BF16, kind="ExternalInput")
    wqkv = nc.dram_tensor("wqkv", [128, NM, KT, 128], BF16,
                          kind="ExternalInput")
    wo = nc.dram_tensor("wo", [128, NQ, DIM], BF16, kind="ExternalInput")
    cc = nc.dram_tensor("cc", [128, R], BF16, kind="ExternalInput")
    ss = nc.dram_tensor("ss", [128, R], BF16, kind="ExternalInput")
    bm = nc.dram_tensor("bm", [128, 128], BF16, kind="ExternalInput")
    out = nc.dram_tensor("out", [R, DIM], BF16, kind="ExternalOutput")


    with tile.TileContext(nc) as tc:
      for _rep in range(reps):
        with (
            tc.tile_pool(name="const", bufs=1) as constp,
            tc.tile_pool(name="qkv", bufs=1) as qkvp,
            tc.tile_pool(name="ctx", bufs=1) as ctxp,
            tc.tile_pool(name="expp", bufs=4) as expp,
            tc.tile_pool(name="dnp", bufs=2) as dnp,
            tc.tile_pool(name="nrm", bufs=2) as nrmp,
            tc.tile_pool(name="scps", bufs=2, space="PSUM") as scps,
            tc.tile_pool(name="cxps", bufs=2, space="PSUM") as cxps,
        ):
            bm_sb = constp.tile([128, 128], BF16)
            ident = constp.tile([128, 128], BF16)
            ones_sb = constp.tile([128, 1], BF16)
            make_identity(nc, ident)
            nc.vector.memset(ones_sb[:], 1.0)

            # persistent activations
            qkT = qkvp.tile([128, NQ + 1, R], BF16)   # roped qT (4 heads) + kT
            v_sb = qkvp.tile([128, R // 128, 128], BF16)  # v natural, row tiles
            ctxT = ctxp.tile([128, NQ, R], BF16)

            # ---------------- attention emission ----------------
            def make_attn_closures(b, j):
                """5 closures for (batch b, q-block j): 4 h-chains (tails
                folded into the next chain's head) + trailing finalize.
                Scores/exp run one kb-PAIR ahead of ctx; off-diagonal pairs
                get a single [128,2,512] exp so Act keeps pace with PE."""
                n = (j + 1) * DIAG
                NP = n // 2
                st = {}    # (h, kb) -> (ex tile, half index, c0)
                sth = {}   # h -> (dn, cx)

                def score_pair(h, p):
                    qh = qkT[:, h, b * S + j * QBLK:b * S + (j + 1) * QBLK]
                    kh = qkT[:, NQ, b * S:(b + 1) * S]
                    sc = scps.tile([128, 2, QBLK], F32, tag="sc")
                    ex = expp.tile([128, 2, QBLK], BF16, tag="ex")
                    info = []
                    for i in range(2):
                        kb = 2 * p + i
                        rel = kb - j * DIAG
                        c0 = rel * KBLK if rel > 0 else 0
                        # fully-masked cols [0:c0) of diagonal blocks skipped
                        nc.tensor.matmul(
                            sc[:, i, c0:], kh[:, kb * KBLK:(kb + 1) * KBLK],
                            qh[:, c0:], start=True, stop=(rel < 0),
                        )
                        if rel >= 0:
                            # fold the causal triangle into the PSUM group:
                            # += ident.T @ tril(-1000) on the diagonal
                            # 128-col window, so exp underflows to 0 there
                            w0 = rel * KBLK
                            nc.tensor.matmul(
                                sc[:, i, w0:w0 + KBLK], ident[:], bm_sb[:],
                                start=False, stop=True,
                            )
                        info.append((kb, rel, c0))
                    if info[0][1] < 0 and info[1][1] < 0:
                        nc.scalar.activation(ex[:, 0:2, :], sc[:, 0:2, :],
                                             AF.Exp)
                    else:
                        for i, (kb, rel, c0) in enumerate(info):
                            nc.scalar.activation(ex[:, i, c0:], sc[:, i, c0:],
                                                 AF.Exp)
                    for i, (kb, rel, c0) in enumerate(info):
                        st[(h, kb)] = (ex, i, c0)

                def dnctx(h, kb):
                    ex, i, c0 = st.pop((h, kb))
                    dn, cx = sth[h]
                    if kb == 0:
                        nc.vector.tensor_copy(dn[:, :], ex[:, 0, :])
                    else:
                        nc.vector.tensor_add(dn[:, c0:], dn[:, c0:],
                                             ex[:, i, c0:])
                    nc.tensor.matmul(
                        cx[:, c0:], v_sb[:, b * ST + kb, :], ex[:, i, c0:],
                        start=(kb == 0), stop=(kb == n - 1),
                    )

                def finalize(h):
                    dn, cx = sth.pop(h)
                    dsp = scps.tile([1, QBLK], F32, tag="sc")
                    nc.tensor.matmul(dsp[:], ones_sb[:], dn[:, :],
                                     start=True, stop=True)
                    rec = nrmp.tile([1, QBLK], F32, tag="rec")
                    recb = nrmp.tile([128, QBLK], F32, tag="recb")
                    nc.vector.reciprocal_approx_fast(rec[:], dsp[:])
                    nc.gpsimd.partition_broadcast(recb[:], rec[:])
                    nc.vector.tensor_mul(
                        ctxT[:, h, b * S + j * QBLK:b * S + (j + 1) * QBLK],
                        cx[:], recb[:],
                    )

                def chain(h):
                    def c():
                        dn = dnp.tile([128, QBLK], BF16, tag="dn")
                        cx = cxps.tile([128, QBLK], F32, tag="cx")
                        sth[h] = (dn, cx)
                        score_pair(h, 0)
                        if h > 0:
                            dnctx(h - 1, n - 2)
                            dnctx(h - 1, n - 1)
                            finalize(h - 1)
                        for p in range(1, NP):
                            score_pair(h, p)
                            dnctx(h, 2 * p - 2)
                            dnctx(h, 2 * p - 1)
                    return c

                def trailer():
                    dnctx(NQ - 1, n - 2)
                    dnctx(NQ - 1, n - 1)
                    finalize(NQ - 1)

                return [chain(h) for h in range(NQ)] + [trailer]

            # ============ phase 1 (QKV projection) + attention ============
            # PSUM: p1 2 + sc 4 + cx 2 = 8 banks.
            with (
                tc.tile_pool(name="wq", bufs=1) as wp,
                tc.tile_pool(name="xin", bufs=6) as xp,
                tc.tile_pool(name="p1ps", bufs=2, space="PSUM") as p1ps,
                tc.tile_pool(name="p1tmp", bufs=2) as p1tmp,
                tc.tile_pool(name="csp", bufs=2) as csp,
            ):
                w_sb = wp.tile([128, NM, KT, 128], BF16)

                xtiles = {}
                cstiles = {}

                def dma_x_cb(cb, first=False):
                    csl = slice(cb * CB, (cb + 1) * CB)
                    KQ = KT // 4
                    xq = [xp.tile([128, KQ, CB], BF16, tag="xcb",
                                  name=f"xq{i}") for i in range(4)]
                    xtiles[cb] = xq
                    cct = csp.tile([128, CB], BF16, tag="cc")
                    sst = csp.tile([128, CB], BF16, tag="ss")
                    cstiles[cb] = (cct, sst)
                    if first:
                        # two parallel HWDGE rings, loaded in need-order.
                        # The Act ring starts ~4us late (ACT table-load
                        # preamble), so the first two w chunks ride the Sync
                        # ring with the early x chunks; the k-head halves go
                        # on Sync too (idle after x), v-head + bm trail on
                        # Act behind the last q-weight chunk
                        for g in range(0, KT, 2):
                            xt, go = xq[g // KQ], g % KQ
                            nc.sync.dma_start(out=xt[:, go:go + 2, :],
                                              in_=xb[cb, :, g:g + 2, :])
                            if g % 4 == 0:
                                # one strided DMA covers all 4 q-head chunks
                                eng = nc.sync if g == 0 else nc.scalar
                                eng.dma_start(
                                    out=w_sb[:, 0:NQ, g:g + 4],
                                    in_=wqkv[:, 0:NQ, g:g + 4])
                            if g == 4:
                                nc.scalar.dma_start(out=cct[:], in_=cc[:, csl])
                                nc.scalar.dma_start(out=sst[:], in_=ss[:, csl])
                        nc.sync.dma_start(out=w_sb[:, NQ, 0:KTH],
                                          in_=wqkv[:, NQ, 0:KTH])
                        nc.sync.dma_start(out=w_sb[:, NQ, KTH:KT],
                                          in_=wqkv[:, NQ, KTH:KT])
                        nc.scalar.dma_start(out=w_sb[:, NQ + 1, 0:KTH],
                                            in_=wqkv[:, NQ + 1, 0:KTH])
                        nc.scalar.dma_start(out=w_sb[:, NQ + 1, KTH:KT],
                                            in_=wqkv[:, NQ + 1, KTH:KT])
                        nc.scalar.dma_start(out=bm_sb[:], in_=bm[:])
                    else:
                        for i in range(4):
                            nc.sync.dma_start(
                                out=xq[i][:],
                                in_=xb[cb, :, i * KQ:(i + 1) * KQ, :])
                        nc.scalar.dma_start(out=cct[:], in_=cc[:, csl])
                        nc.scalar.dma_start(out=sst[:], in_=ss[:, csl])

                def rope_evict(m, ps, cct, sst, csl):
                    # RoPE fused into eviction (even|odd permuted):
                    # out = t0*cc + swap_halves(t0)*ss, with t0 = ps evicted
                    # to SBUF first so the PSUM bank frees after one op
                    t0 = p1tmp.tile([128, CB], BF16, tag="t0")
                    nc.vector.tensor_copy(t0[:], ps[:])
                    t2 = p1tmp.tile([128, CB], BF16, tag="t2")
                    # ss is host-laid-out so each mul's SBUF inputs share a
                    # base partition: ss[64:128] = -sin, ss[0:64] = +sin
                    nc.vector.tensor_mul(t2[0:64, :], t0[64:128, :], sst[64:128, :])
                    nc.vector.tensor_mul(t2[64:128, :], t0[0:64, :], sst[0:64, :])
                    dst = qkT[:, m, csl]
                    nc.vector.tensor_mul(dst, t0[:], cct[:])
                    nc.vector.tensor_add(dst, dst, t2[:])

                def emit_v_natural(xq, cb, KQ):
                    """v projected directly into natural [kpos, hd] layout:
                    x-tile is the stationary operand (rows -> out partitions),
                    wv the moving one; one DVE evict lands all 4 row-tiles."""
                    vps = p1ps.tile([128, DIAG, 128], F32, tag="p1")
                    for t in range(DIAG):
                        for kt in range(KT):
                            nc.tensor.matmul(
                                vps[:, t, :],
                                xq[kt // KQ][:, kt % KQ, t * 128:(t + 1) * 128],
                                w_sb[:, NQ + 1, kt, :],
                                start=(kt == 0), stop=(kt == KT - 1),
                            )
                    nc.vector.tensor_copy(
                        v_sb[:, cb * DIAG:(cb + 1) * DIAG, :], vps[:])

                def emit_qkv_cb0():
                    """cb 0, kt-major in two passes (m0-3, then k + v re-
                    reading x from SBUF) so PE keeps pace with the DMA
                    stream and the borrowed attention PSUM frees early."""
                    csl = slice(0, CB)
                    xq = xtiles.pop(0)
                    KQ = KT // 4
                    cct, sst = cstiles.pop(0)
                    sc_a = scps.tile([128, 2, CB], F32, tag="sc", name="sc_a")
                    pss = [p1ps.tile([128, CB], F32, tag="p1", name="ps0"),
                           p1ps.tile([128, CB], F32, tag="p1", name="ps1"),
                           sc_a[:, 0, :], sc_a[:, 1, :]]
                    for kt in range(KT):
                        xsrc = xq[kt // KQ]
                        for m in range(NQ):
                            nc.tensor.matmul(
                                pss[m], w_sb[:, m, kt, :],
                                xsrc[:, kt % KQ, :],
                                start=(kt == 0), stop=(kt == KT - 1),
                            )
                    for m in range(NQ):
                        rope_evict(m, pss[m], cct, sst, csl)
                    dma_x_cb(1)
                    ps4 = p1ps.tile([128, CB], F32, tag="p1", name="ps4")
                    for kt in range(KT):
                        nc.tensor.matmul(
                            ps4[:], w_sb[:, NQ, kt, :],
                            xq[kt // KQ][:, kt % KQ, :],
                            start=(kt == 0), stop=(kt == KT - 1),
                        )
                    rope_evict(NQ, ps4, cct, sst, csl)
                    emit_v_natural(xq, 0, KQ)

                def emit_qkv_cb(cb, closures):
                    csl = slice(cb * CB, (cb + 1) * CB)
                    xq = xtiles.pop(cb)
                    KQ = KT // 4
                    cct, sst = cstiles.pop(cb)
                    ci = 0
                    for m in range(NM):
                        if m < NQ + 1:
                            ps = p1ps.tile([128, CB], F32, tag="p1")
                            for kt in range(KT):
                                xsrc = xq[kt // KQ]
                                nc.tensor.matmul(
                                    ps[:], w_sb[:, m, kt, :],
                                    xsrc[:, kt % KQ, :],
                                    start=(kt == 0), stop=(kt == KT - 1),
                                )
                            rope_evict(m, ps, cct, sst, csl)
                        else:
                            emit_v_natural(xq, cb, KQ)
                        if ci < len(closures):
                            closures[ci]()
                            ci += 1
                        if m == 0 and cb + 1 < NCB:
                            dma_x_cb(cb + 1)
                    while ci < len(closures):
                        closures[ci]()
                        ci += 1

                dma_x_cb(0, first=True)
                # warm the PE HAM clock gate while the first x/w chunks are
                # in flight: ~24 dummy transposes keep the activity window
                # busy so the real chains start at 2.4 GHz instead of 1.2
                wps = p1ps.tile([128, 128], BF16, tag="p1", name="warm")
                for _ in range(24):
                    nc.tensor.transpose(wps[:], ident[:], ident[:])
                # cb -> attention chunk interleaved into its m-chains
                attn_sched = {1: (0, 0), 2: (0, 1), 3: (0, 2), 4: (0, 3),
                              5: (1, 0), 6: (1, 1), 7: (1, 2)}
                emit_qkv_cb0()
                for cb in range(1, NCB):
                    cls = []
                    if cb in attn_sched:
                        ab, aj = attn_sched[cb]
                        cls = make_attn_closures(ab, aj)
                    emit_qkv_cb(cb, cls)

            # ======== tail: attention (b1, j3) + output projection ========
            # PSUM: sc 3 + cx 2 + p3 3 = 8 banks.
            with (
                tc.tile_pool(name="wo", bufs=1) as wop,
                tc.tile_pool(name="p3ps", bufs=2, space="PSUM") as p3ps,
                tc.tile_pool(name="p3o", bufs=4) as p3o,
            ):
                wo_sb = wop.tile([128, NQ, DIM], BF16)
                nc.sync.dma_start(out=wo_sb[:, :, 0:DIM // 2],
                                  in_=wo[:, :, 0:DIM // 2])
                nc.sync.dma_start(out=wo_sb[:, :, DIM // 2:],
                                  in_=wo[:, :, DIM // 2:])

                # post-attention, phase-3 psum rotates through ALL pools
                # (sc/cx idle once (b1,j3) is done) for deep pipelining
                _pend = []
                _cyc = {"i": 0, "full": False}

                def p3_psum():
                    if _pend:
                        return _pend.pop(0)
                    if not _cyc["full"]:
                        return p3ps.tile([128, NBLK], F32, tag="p3",
                                         name="p3t")
                    k = _cyc["i"] % 4
                    _cyc["i"] += 1
                    if k in (0, 1):
                        return p3ps.tile([128, NBLK], F32, tag="p3",
                                         name="p3t")
                    if k == 2:
                        t = scps.tile([128, 2, QBLK], F32, tag="sc",
                                      name="p3sc")
                        _pend.append(t[:, 1, :])
                        return t[:, 0, :]
                    return cxps.tile([128, QBLK], F32, tag="cx", name="p3cx")

                def p3_block(r, np_, fine=False):
                    """Two adjacent NBLK chunks -> one [128, 2*NBLK] store
                    (or two finer stores for the last jobs so the drain
                    tail after the final matmul is short)."""
                    ob = p3o.tile([128, 2 * NBLK], BF16, tag="ob")
                    for half in range(2):
                        n = 2 * np_ + half
                        ps = p3_psum()
                        for h in range(NQ):
                            nc.tensor.matmul(
                                ps[:],
                                ctxT[:, h, r * 128:(r + 1) * 128],
                                wo_sb[:, h, n * NBLK:(n + 1) * NBLK],
                                start=(h == 0), stop=(h == NQ - 1),
                            )
                        dst = ob[:, half * NBLK:(half + 1) * NBLK]
                        # while (b1,j3) attention is live, Act's strict FIFO
                        # must stay clear for exp -> evict on DVE only;
                        # afterwards Act (it is otherwise idle and sits
                        # closer to PSUM)
                        if _cyc["full"]:
                            nc.scalar.copy(dst, ps[:])
                        else:
                            nc.vector.tensor_copy(dst, ps[:])
                        if fine:
                            nc.sync.dma_start(
                                out=out[r * 128:(r + 1) * 128,
                                        n * NBLK:(n + 1) * NBLK],
                                in_=dst,
                            )
                    if not fine:
                        nc.sync.dma_start(
                            out=out[r * 128:(r + 1) * 128,
                                    2 * np_ * NBLK:2 * (np_ + 1) * NBLK],
                            in_=ob[:],
                        )

                # blocks for rows whose ctxT is ready before (b1,j3) finishes
                jobs = [(r, np_) for r in range(R // 128 - 4)
                        for np_ in range(NN // 2)]
                jobs += [(r, np_) for r in range(R // 128 - 4, R // 128)
                         for np_ in range(NN // 2)]
                cls = make_attn_closures(1, 3)
                ji = 0
                for c in cls:
                    c()
                    for _ in range(6):
                        if ji < len(jobs):
                            p3_block(*jobs[ji])
                            ji += 1
                _cyc["full"] = True  # attention done: rotate all psum pools
                while ji < len(jobs):
                    p3_block(*jobs[ji], fine=(ji >= len(jobs) - 4))
                    ji += 1

    nc.compile()
    return nc


# ---------------- host-side sharding ----------------

_EO_PERM = np.concatenate([np.arange(0, 128, 2), np.arange(1, 128, 2)])


def shard_inputs(cfg: Cfg, x, wq, wk, wv, wo, freqs_cos, freqs_sin, mask,
                 n_cores: int):
    """Build per-core input maps (numpy, bf16)."""
    bf = ml_dtypes.bfloat16
    B, S, DIM, NQ, HD = cfg.B, cfg.S, cfg.DIM, cfg.NQ, cfg.HD
    R = cfg.R
    x2 = np.asarray(x, np.float32).reshape(R, DIM)
    KT, CB = DIM // 128, 512
    NCB = R // CB
    # [NCB, 128, KT, CB]: xb[cb, p, kt, r] = x.T[kt*128+p, cb*CB+r]
    xb = np.ascontiguousarray(
        x2.T.reshape(KT, 128, NCB, CB).transpose(2, 1, 0, 3)).astype(bf)

    scale = 1.0 / math.sqrt(HD)
    wq = np.asarray(wq, np.float32) * scale
    wk = np.asarray(wk, np.float32)
    wv = np.asarray(wv, np.float32)
    wo = np.asarray(wo, np.float32)

    cosT = np.asarray(freqs_cos, np.float32).T  # [64, S]
    sinT = np.asarray(freqs_sin, np.float32).T
    cc1 = np.concatenate([cosT, cosT], axis=0)          # [128, S]
    ss1 = np.concatenate([sinT, -sinT], axis=0)
    cc = np.tile(cc1, (1, B)).astype(bf)                # [128, R]
    ss = np.tile(ss1, (1, B)).astype(bf)

    # additive causal bias for diagonal 128-blocks, applied on PE via
    # ident.T @ bm: bm[p, q] = -1000 where k-pos p > q-pos q (strictly
    # above the diagonal), 0 elsewhere; exp then underflows to 0 there
    bm = np.tril(np.full((128, 128), -1000.0, np.float32), -1).astype(bf)

    in_maps = []
    for c in range(n_cores):
        qcols = []
        for i in range(NQ):
            h = c * NQ + i
            qcols.append(wq[:, h * HD:(h + 1) * HD][:, _EO_PERM])
        kcol = wk[:, c * HD:(c + 1) * HD][:, _EO_PERM]
        vcol = wv[:, c * HD:(c + 1) * HD]
        wqkv_c = np.concatenate(qcols + [kcol, vcol], axis=1)
        # [128, NM, KT, 128]: w[p, m, kt, j] = wqkv[kt*128+p, m*128+j]
        wqkv_c = np.ascontiguousarray(
            wqkv_c.reshape(KT, 128, 6, 128).transpose(1, 2, 0, 3)).astype(bf)
        wo_c = wo[c * NQ * HD:(c + 1) * NQ * HD, :]
        # [128, NQ, DIM]: wo[p, h, n] = wo_c[h*128+p, n]
        wo_c = np.ascontiguousarray(
            wo_c.reshape(NQ, 128, DIM).transpose(1, 0, 2)).astype(bf)
        in_maps.append({
            "xb": xb, "wqkv": wqkv_c, "wo": wo_c,
            "cc": cc, "ss": ss, "bm": bm,
        })
    return in_maps


_NC_CACHE = {}


def _get_nc(cfg: Cfg):
    if cfg not in _NC_CACHE:
        _NC_CACHE[cfg] = build_nc(cfg)
    return _NC_CACHE[cfg]


def kernel(x, wq, wk, wv, wo, freqs_cos, freqs_sin, mask, start_pos=0,
           **_ignored):
    from concourse.bass_utils import run_bass_kernel_spmd

    cfg = Cfg()
    nc = _get_nc(cfg)
    in_maps = shard_inputs(cfg, x, wq, wk, wv, wo, freqs_cos, freqs_sin, mask,
                           n_cores=8)
    res = run_bass_kernel_spmd(nc, in_maps, core_ids=list(range(8)))
    acc = np.zeros((cfg.R, cfg.DIM), np.float32)
    for c in range(8):
        acc += res.results[c]["out"].astype(np.float32)
    return acc.reshape(cfg.B, cfg.S, cfg.DIM)


# revision 24
# speedup vs baseline: 1.8306x; 1.8306x over previous
"""Trainium2 Bass kernel for GQA attention prefill (nn_Attention_60593398612481).

Full-input contract: kernel(**inputs) takes the unsharded inputs and returns
the full [B, S, DIM] fp32 output. Internally: tensor-parallel across heads on
8 NeuronCores (q-heads 4c..4c+3 + kv-head c on core c; wo row-sharded), each
core computes a full-shape partial of the output projection, host sums the 8
partials (row-parallel "all-reduce" realized at gather time).

Assumes the mask input is the standard causal mask (0 on/below diagonal,
-1e9 above) as produced by the reference setup_inputs().

Schedule (single instruction stream, engines overlap via Tile deps):
- cb 0 runs kt-major in two passes (m0-3, then k + v re-reading x from SBUF)
  so PE keeps pace with the fine-chunked startup DMA stream (no single wait
  crosses the ~3us clock-rewarm threshold); from cb 1 onward, attention
  chunks are interleaved between projection m-chains so their exp/softmax
  chains hide under projection matmuls:
    cb1+attn(b0,j0) ... cb4+(b0,j3), cb5+(b1,j0) ... cb7+(b1,j2)
- Tail: attention (b1,j3) interleaved with the output projection blocks;
  once attention is done its PSUM banks join the projection rotation.
- Attention is emitted j-outer / h-inner; scores+exp run one kb-PAIR ahead
  of the ctx matmuls (one [128,2,512] exp per off-diagonal pair keeps Act
  ahead of PE), chain tails fold into the next chain's head, and the fully
  masked columns of diagonal blocks are skipped in scores/exp.
- Causal masking of diagonal 128-blocks is folded into the scores PSUM
  accumulation as one extra PE matmul (ident.T @ tril(-1000)) so exp of the
  masked region underflows to 0 -- no Pool/DVE op sits between exp and the
  ctx matmul (the old Pool triangle-mask was the dominant PE stall).
- x/wqkv/wo are host-blocked so every DMA reads long contiguous
  per-partition lines (x 16KB, w >=1KB); x streams in quarter-cb tiles
  (6-slot ring) so prefetch slots release 4x finer than the consuming
  chain; x comes in pre-transposed, weights column-sharded, so qT/kT leave
  the QKV matmul with head-dim on partitions; RoPE (even|odd permuted) is
  fused into the PSUM eviction on DVE (evict-first so the PSUM bank frees
  after one op); v is projected directly into natural [kpos, hd] layout by
  swapping the matmul operand roles (x-tile stationary, wv moving), so no
  transposes or Act copies sit on the v path.
- Softmax runs without max-subtraction (|s| <~ 10); denominator = ones-matmul
  partition sum; reciprocal via the fast custom-DVE approx (~5x faster than
  the iterative-divide reciprocal, well within tolerance).
- Engine split: exp on Act, dn-accumulate + rope + normalize + v-evict on
  DVE, recip-broadcast on Pool, psum evictions DVE while attention is live,
  Act afterwards.
"""

import math
from dataclasses import dataclass

import numpy as np
import ml_dtypes

import concourse.bass as bass
import concourse.mybir as mybir
import concourse.tile as tile
from concourse import bacc
from concourse.masks import make_identity

BF16 = mybir.dt.bfloat16
F32 = mybir.dt.float32
AF = mybir.ActivationFunctionType


@dataclass(frozen=True)
class Cfg:
    B: int = 2
    S: int = 2048
    DIM: int = 4096
    NQ: int = 4        # q heads per core
    HD: int = 128
    CB: int = 512      # phase-1 column block (rows of x)
    QBLK: int = 512    # attention q block (PSUM bank)
    KBLK: int = 128    # attention k block (partition dim)
    NBLK: int = 512    # phase-3 out-dim block

    @property
    def R(self):
        return self.B * self.S

    @property
    def KT(self):
        return self.DIM // 128

    @property
    def NM(self):
        return self.NQ + 2  # q heads + k + v


def build_nc(cfg: Cfg, reps: int = 1):
    """Build the single-core Bass program (SPMD: same program on 8 cores)."""
    nc = bacc.Bacc("TRN2", target_bir_lowering=False)
    B, S, DIM, NQ = cfg.B, cfg.S, cfg.DIM, cfg.NQ
    R, KT, NM = cfg.R, cfg.KT, cfg.NM
    CB, QBLK, KBLK = cfg.CB, cfg.QBLK, cfg.KBLK
    NBLK = cfg.NBLK
    NCB = R // CB
    ST = S // 128          # seq row-tiles per batch
    DIAG = QBLK // KBLK    # diagonal k-blocks per q-block
    NN = DIM // NBLK
    KTH = KT // 2

    # host-blocked layouts: every DMA reads long contiguous per-partition
    # lines (x: 16KB, w: >=1KB) instead of 1KB/256B strided runs
    xb = nc.dram_tensor("xb", [NCB, 128, KT, CB], BF16, kind="ExternalInput")
    wqkv = nc.dram_tensor("wqkv", [128, NM, KT, 128], BF16,
                          kind="ExternalInput")
    wo = nc.dram_tensor("wo", [128, NQ, DIM], BF16, kind="ExternalInput")
    cc = nc.dram_tensor("cc", [128, R], BF16, kind="ExternalInput")
    ss = nc.dram_tensor("ss", [128, R], BF16, kind="ExternalInput")
    bm = nc.dram_tensor("bm", [128, 128], BF16, kind="ExternalInput")
    out = nc.dram_tensor("out", [R, DIM], BF16, kind="ExternalOutput")


    with tile.TileContext(nc) as tc:
      for _rep in range(reps):
        with (
            tc.tile_pool(name="const", bufs=1) as constp,
            tc.tile_pool(name="qkv", bufs=1) as qkvp,
            tc.tile_pool(name="ctx", bufs=1) as ctxp,
            tc.tile_pool(name="expp", bufs=4) as expp,
            tc.tile_pool(name="dnp", bufs=2) as dnp,
            tc.tile_pool(name="nrm", bufs=2) as nrmp,
            tc.tile_pool(name="scps", bufs=2, space="PSUM") as scps,
            tc.tile_pool(name="cxps", bufs=2, space="PSUM") as cxps,
        ):
            bm_sb = constp.tile([128, 128], BF16)
            ident = constp.tile([128, 128], BF16)
            ones_sb = constp.tile([128, 1], BF16)
            make_identity(nc, ident)
            nc.vector.memset(ones_sb[:], 1.0)

            # persistent activations
            qkT = qkvp.tile([128, NQ + 1, R], BF16)   # roped qT (4 heads) + kT
            v_sb = qkvp.tile([128, R // 128, 128], BF16)  # v natural, row tiles
            ctxT = ctxp.tile([128, NQ, R], BF16)

            # ---------------- attention emission ----------------
            def make_attn_closures(b, j, filler=None):
                """5 closures for (batch b, q-block j): 4 h-chains (tails
                folded into the next chain's head) + trailing finalize.
                Scores/exp run one kb-PAIR ahead of ctx; off-diagonal pairs
                get a single [128,2,512] exp so Act keeps pace with PE.
                `filler` (tail only) emits extra PE work inside the chain
                every few pairs so Act's exp latency stays hidden during
                pure-attention stretches."""
                n = (j + 1) * DIAG
                NP = n // 2
                st = {}    # (h, kb) -> (ex tile, half index, c0)
                sth = {}   # h -> (dn, cx)

                def score_pair(h, p):
                    qh = qkT[:, h, b * S + j * QBLK:b * S + (j + 1) * QBLK]
                    kh = qkT[:, NQ, b * S:(b + 1) * S]
                    sc = scps.tile([128, 2, QBLK], F32, tag="sc")
                    ex = expp.tile([128, 2, QBLK], BF16, tag="ex")
                    info = []
                    for i in range(2):
                        kb = 2 * p + i
                        rel = kb - j * DIAG
                        c0 = rel * KBLK if rel > 0 else 0
                        # fully-masked cols [0:c0) of diagonal blocks skipped
                        nc.tensor.matmul(
                            sc[:, i, c0:], kh[:, kb * KBLK:(kb + 1) * KBLK],
                            qh[:, c0:], start=True, stop=(rel < 0),
                        )
                        if rel >= 0:
                            # fold the causal triangle into the PSUM group:
                            # += ident.T @ tril(-1000) on the diagonal
                            # 128-col window, so exp underflows to 0 there
                            w0 = rel * KBLK
                            nc.tensor.matmul(
                                sc[:, i, w0:w0 + KBLK], ident[:], bm_sb[:],
                                start=False, stop=True,
                            )
                        info.append((kb, rel, c0))
                    if info[0][1] < 0 and info[1][1] < 0:
                        nc.scalar.activation(ex[:, 0:2, :], sc[:, 0:2, :],
                                             AF.Exp)
                    else:
                        for i, (kb, rel, c0) in enumerate(info):
                            nc.scalar.activation(ex[:, i, c0:], sc[:, i, c0:],
                                                 AF.Exp)
                    for i, (kb, rel, c0) in enumerate(info):
                        st[(h, kb)] = (ex, i, c0)

                def dnctx(h, kb):
                    ex, i, c0 = st.pop((h, kb))
                    dn, cx = sth[h]
                    if kb == 0:
                        nc.vector.tensor_copy(dn[:, :], ex[:, 0, :])
                    else:
                        nc.vector.tensor_add(dn[:, c0:], dn[:, c0:],
                                             ex[:, i, c0:])
                    nc.tensor.matmul(
                        cx[:, c0:], v_sb[:, b * ST + kb, :], ex[:, i, c0:],
                        start=(kb == 0), stop=(kb == n - 1),
                    )

                def finalize(h):
                    dn, cx = sth.pop(h)
                    dsp = scps.tile([1, QBLK], F32, tag="sc")
                    nc.tensor.matmul(dsp[:], ones_sb[:], dn[:, :],
                                     start=True, stop=True)
                    rec = nrmp.tile([1, QBLK], F32, tag="rec")
                    recb = nrmp.tile([128, QBLK], F32, tag="recb")
                    nc.vector.reciprocal_approx_fast(rec[:], dsp[:])
                    nc.gpsimd.partition_broadcast(recb[:], rec[:])
                    nc.vector.tensor_mul(
                        ctxT[:, h, b * S + j * QBLK:b * S + (j + 1) * QBLK],
                        cx[:], recb[:],
                    )

                def chain(h):
                    def c():
                        dn = dnp.tile([128, QBLK], BF16, tag="dn")
                        cx = cxps.tile([128, QBLK], F32, tag="cx")
                        sth[h] = (dn, cx)
                        score_pair(h, 0)
                        if h > 0:
                            dnctx(h - 1, n - 2)
                            dnctx(h - 1, n - 1)
                            finalize(h - 1)
                        for p in range(1, NP):
                            score_pair(h, p)
                            dnctx(h, 2 * p - 2)
                            dnctx(h, 2 * p - 1)
                            if filler is not None and p % 3 == 0:
                                filler()
                    return c

                def trailer():
                    dnctx(NQ - 1, n - 2)
                    dnctx(NQ - 1, n - 1)
                    finalize(NQ - 1)

                return [chain(h) for h in range(NQ)] + [trailer]

            # ============ phase 1 (QKV projection) + attention ============
            # PSUM: p1 2 + sc 4 + cx 2 = 8 banks.
            with (
                tc.tile_pool(name="wq", bufs=1) as wp,
                tc.tile_pool(name="xin", bufs=6) as xp,
                tc.tile_pool(name="p1ps", bufs=2, space="PSUM") as p1ps,
                tc.tile_pool(name="p1tmp", bufs=2) as p1tmp,
                tc.tile_pool(name="csp", bufs=2) as csp,
            ):
                w_sb = wp.tile([128, NM, KT, 128], BF16)

                xtiles = {}
                cstiles = {}

                def dma_x_cb(cb, first=False):
                    csl = slice(cb * CB, (cb + 1) * CB)
                    KQ = KT // 4
                    xq = [xp.tile([128, KQ, CB], BF16, tag="xcb",
                                  name=f"xq{i}") for i in range(4)]
                    xtiles[cb] = xq
                    cct = csp.tile([128, CB], BF16, tag="cc")
                    sst = csp.tile([128, CB], BF16, tag="ss")
                    cstiles[cb] = (cct, sst)
                    if first:
                        # two parallel HWDGE rings, loaded in need-order.
                        # The Act ring starts ~4us late (ACT table-load
                        # preamble), so the first two w chunks ride the Sync
                        # ring with the early x chunks; the k-head halves go
                        # on Sync too (idle after x), v-head + bm trail on
                        # Act behind the last q-weight chunk
                        for g in range(0, KT, 2):
                            xt, go = xq[g // KQ], g % KQ
                            nc.sync.dma_start(out=xt[:, go:go + 2, :],
                                              in_=xb[cb, :, g:g + 2, :])
                            if g % 4 == 0:
                                # one strided DMA covers all 4 q-head chunks
                                eng = nc.sync if g == 0 else nc.scalar
                                eng.dma_start(
                                    out=w_sb[:, 0:NQ, g:g + 4],
                                    in_=wqkv[:, 0:NQ, g:g + 4])
                            if g == 4:
                                nc.scalar.dma_start(out=cct[:], in_=cc[:, csl])
                                nc.scalar.dma_start(out=sst[:], in_=ss[:, csl])
                        nc.sync.dma_start(out=w_sb[:, NQ, 0:KTH],
                                          in_=wqkv[:, NQ, 0:KTH])
                        nc.sync.dma_start(out=w_sb[:, NQ, KTH:KT],
                                          in_=wqkv[:, NQ, KTH:KT])
                        nc.scalar.dma_start(out=w_sb[:, NQ + 1, 0:KTH],
                                            in_=wqkv[:, NQ + 1, 0:KTH])
                        nc.scalar.dma_start(out=w_sb[:, NQ + 1, KTH:KT],
                                            in_=wqkv[:, NQ + 1, KTH:KT])
                        nc.scalar.dma_start(out=bm_sb[:], in_=bm[:])
                    else:
                        for i in range(4):
                            nc.sync.dma_start(
                                out=xq[i][:],
                                in_=xb[cb, :, i * KQ:(i + 1) * KQ, :])
                        nc.scalar.dma_start(out=cct[:], in_=cc[:, csl])
                        nc.scalar.dma_start(out=sst[:], in_=ss[:, csl])

                def rope_evict(m, ps, cct, sst, csl):
                    # RoPE fused into eviction (even|odd permuted):
                    # out = t0*cc + swap_halves(t0)*ss, with t0 = ps evicted
                    # to SBUF first so the PSUM bank frees after one op
                    t0 = p1tmp.tile([128, CB], BF16, tag="t0")
                    nc.vector.tensor_copy(t0[:], ps[:])
                    t2 = p1tmp.tile([128, CB], BF16, tag="t2")
                    # ss is host-laid-out so each mul's SBUF inputs share a
                    # base partition: ss[64:128] = -sin, ss[0:64] = +sin
                    nc.vector.tensor_mul(t2[0:64, :], t0[64:128, :], sst[64:128, :])
                    nc.vector.tensor_mul(t2[64:128, :], t0[0:64, :], sst[0:64, :])
                    dst = qkT[:, m, csl]
                    nc.vector.tensor_mul(dst, t0[:], cct[:])
                    nc.vector.tensor_add(dst, dst, t2[:])

                def emit_v_natural(xq, cb, KQ):
                    """v projected directly into natural [kpos, hd] layout:
                    x-tile is the stationary operand (rows -> out partitions),
                    wv the moving one; one DVE evict lands all 4 row-tiles."""
                    vps = p1ps.tile([128, DIAG, 128], F32, tag="p1")
                    for t in range(DIAG):
                        for kt in range(KT):
                            nc.tensor.matmul(
                                vps[:, t, :],
                                xq[kt // KQ][:, kt % KQ, t * 128:(t + 1) * 128],
                                w_sb[:, NQ + 1, kt, :],
                                start=(kt == 0), stop=(kt == KT - 1),
                            )
                    nc.vector.tensor_copy(
                        v_sb[:, cb * DIAG:(cb + 1) * DIAG, :], vps[:])

                def emit_qkv_cb0():
                    """cb 0, kt-major in two passes (m0-3, then k + v re-
                    reading x from SBUF) so PE keeps pace with the DMA
                    stream and the borrowed attention PSUM frees early."""
                    csl = slice(0, CB)
                    xq = xtiles.pop(0)
                    KQ = KT // 4
                    cct, sst = cstiles.pop(0)
                    sc_a = scps.tile([128, 2, CB], F32, tag="sc", name="sc_a")
                    pss = [p1ps.tile([128, CB], F32, tag="p1", name="ps0"),
                           p1ps.tile([128, CB], F32, tag="p1", name="ps1"),
                           sc_a[:, 0, :], sc_a[:, 1, :]]
                    for kt in range(KT):
                        xsrc = xq[kt // KQ]
                        for m in range(NQ):
                            nc.tensor.matmul(
                                pss[m], w_sb[:, m, kt, :],
                                xsrc[:, kt % KQ, :],
                                start=(kt == 0), stop=(kt == KT - 1),
                            )
                    for m in range(NQ):
                        rope_evict(m, pss[m], cct, sst, csl)
                    dma_x_cb(1)
                    ps4 = p1ps.tile([128, CB], F32, tag="p1", name="ps4")
                    for kt in range(KT):
                        nc.tensor.matmul(
                            ps4[:], w_sb[:, NQ, kt, :],
                            xq[kt // KQ][:, kt % KQ, :],
                            start=(kt == 0), stop=(kt == KT - 1),
                        )
                    rope_evict(NQ, ps4, cct, sst, csl)
                    emit_v_natural(xq, 0, KQ)

                def emit_qkv_cb(cb, closures):
                    csl = slice(cb * CB, (cb + 1) * CB)
                    xq = xtiles.pop(cb)
                    KQ = KT // 4
                    cct, sst = cstiles.pop(cb)
                    ci = 0
                    for m in range(NM):
                        if m < NQ + 1:
                            ps = p1ps.tile([128, CB], F32, tag="p1")
                            for kt in range(KT):
                                xsrc = xq[kt // KQ]
                                nc.tensor.matmul(
                                    ps[:], w_sb[:, m, kt, :],
                                    xsrc[:, kt % KQ, :],
                                    start=(kt == 0), stop=(kt == KT - 1),
                                )
                            rope_evict(m, ps, cct, sst, csl)
                        else:
                            emit_v_natural(xq, cb, KQ)
                        if ci < len(closures):
                            closures[ci]()
                            ci += 1
                        if m == 0 and cb + 1 < NCB:
                            dma_x_cb(cb + 1)
                    while ci < len(closures):
                        closures[ci]()
                        ci += 1

                dma_x_cb(0, first=True)
                # warm the PE HAM clock gate while the first x/w chunks are
                # in flight: ~24 dummy transposes keep the activity window
                # busy so the real chains start at 2.4 GHz instead of 1.2
                wps = p1ps.tile([128, 128], BF16, tag="p1", name="warm")
                for _ in range(24):
                    nc.tensor.transpose(wps[:], ident[:], ident[:])
                # cb -> attention chunk interleaved into its m-chains
                attn_sched = {1: (0, 0), 2: (0, 1), 3: (0, 2), 4: (0, 3),
                              5: (1, 0), 6: (1, 1), 7: (1, 2)}
                emit_qkv_cb0()
                for cb in range(1, NCB):
                    cls = []
                    if cb in attn_sched:
                        ab, aj = attn_sched[cb]
                        cls = make_attn_closures(ab, aj)
                    emit_qkv_cb(cb, cls)

            # ======== tail: attention (b1, j3) + output projection ========
            # PSUM: sc 3 + cx 2 + p3 3 = 8 banks.
            with (
                tc.tile_pool(name="wo", bufs=1) as wop,
                tc.tile_pool(name="p3ps", bufs=2, space="PSUM") as p3ps,
                tc.tile_pool(name="p3o", bufs=4) as p3o,
            ):
                wo_sb = wop.tile([128, NQ, DIM], BF16)
                nc.sync.dma_start(out=wo_sb[:, :, 0:DIM // 2],
                                  in_=wo[:, :, 0:DIM // 2])
                nc.sync.dma_start(out=wo_sb[:, :, DIM // 2:],
                                  in_=wo[:, :, DIM // 2:])

                # post-attention, phase-3 psum rotates through ALL pools
                # (sc/cx idle once (b1,j3) is done) for deep pipelining
                _pend = []
                _cyc = {"i": 0, "full": False}

                def p3_psum():
                    if _pend:
                        return _pend.pop(0)
                    if not _cyc["full"]:
                        return p3ps.tile([128, NBLK], F32, tag="p3",
                                         name="p3t")
                    k = _cyc["i"] % 4
                    _cyc["i"] += 1
                    if k in (0, 1):
                        return p3ps.tile([128, NBLK], F32, tag="p3",
                                         name="p3t")
                    if k == 2:
                        t = scps.tile([128, 2, QBLK], F32, tag="sc",
                                      name="p3sc")
                        _pend.append(t[:, 1, :])
                        return t[:, 0, :]
                    return cxps.tile([128, QBLK], F32, tag="cx", name="p3cx")

                def p3_block(r, np_, fine=False):
                    """Two adjacent NBLK chunks -> one [128, 2*NBLK] store
                    (or two finer stores for the last jobs so the drain
                    tail after the final matmul is short)."""
                    ob = p3o.tile([128, 2 * NBLK], BF16, tag="ob")
                    for half in range(2):
                        n = 2 * np_ + half
                        ps = p3_psum()
                        for h in range(NQ):
                            nc.tensor.matmul(
                                ps[:],
                                ctxT[:, h, r * 128:(r + 1) * 128],
                                wo_sb[:, h, n * NBLK:(n + 1) * NBLK],
                                start=(h == 0), stop=(h == NQ - 1),
                            )
                        dst = ob[:, half * NBLK:(half + 1) * NBLK]
                        # while (b1,j3) attention is live, Act's strict FIFO
                        # must stay clear for exp -> evict on DVE only;
                        # afterwards Act (it is otherwise idle and sits
                        # closer to PSUM); final fine jobs split across both
                        # engines so the drain tail is short
                        if fine and half == 1:
                            nc.vector.tensor_copy(dst, ps[:])
                        elif _cyc["full"]:
                            nc.scalar.copy(dst, ps[:])
                        else:
                            nc.vector.tensor_copy(dst, ps[:])
                        if fine:
                            nc.sync.dma_start(
                                out=out[r * 128:(r + 1) * 128,
                                        n * NBLK:(n + 1) * NBLK],
                                in_=dst,
                            )
                    if not fine:
                        nc.sync.dma_start(
                            out=out[r * 128:(r + 1) * 128,
                                    2 * np_ * NBLK:2 * (np_ + 1) * NBLK],
                            in_=ob[:],
                        )

                # blocks for rows whose ctxT is ready before (b1,j3) finishes
                jobs = [(r, np_) for r in range(R // 128 - 4)
                        for np_ in range(NN // 2)]
                jobs += [(r, np_) for r in range(R // 128 - 4, R // 128)
                         for np_ in range(NN // 2)]
                st = {"ji": 0}

                def do_job():
                    if st["ji"] < len(jobs):
                        p3_block(*jobs[st["ji"]])
                        st["ji"] += 1

                cls = make_attn_closures(1, 3, filler=do_job)
                for c in cls:
                    c()
                    for _ in range(6):
                        do_job()
                _cyc["full"] = True  # attention done: rotate all psum pools
                while st["ji"] < len(jobs):
                    p3_block(*jobs[st["ji"]], fine=(st["ji"] >= len(jobs) - 4))
                    st["ji"] += 1

    nc.compile()
    return nc


# ---------------- host-side sharding ----------------

_EO_PERM = np.concatenate([np.arange(0, 128, 2), np.arange(1, 128, 2)])


def shard_inputs(cfg: Cfg, x, wq, wk, wv, wo, freqs_cos, freqs_sin, mask,
                 n_cores: int):
    """Build per-core input maps (numpy, bf16)."""
    bf = ml_dtypes.bfloat16
    B, S, DIM, NQ, HD = cfg.B, cfg.S, cfg.DIM, cfg.NQ, cfg.HD
    R = cfg.R
    x2 = np.asarray(x, np.float32).reshape(R, DIM)
    KT, CB = DIM // 128, 512
    NCB = R // CB
    # [NCB, 128, KT, CB]: xb[cb, p, kt, r] = x.T[kt*128+p, cb*CB+r]
    xb = np.ascontiguousarray(
        x2.T.reshape(KT, 128, NCB, CB).transpose(2, 1, 0, 3)).astype(bf)

    scale = 1.0 / math.sqrt(HD)
    wq = np.asarray(wq, np.float32) * scale
    wk = np.asarray(wk, np.float32)
    wv = np.asarray(wv, np.float32)
    wo = np.asarray(wo, np.float32)

    cosT = np.asarray(freqs_cos, np.float32).T  # [64, S]
    sinT = np.asarray(freqs_sin, np.float32).T
    cc1 = np.concatenate([cosT, cosT], axis=0)          # [128, S]
    ss1 = np.concatenate([sinT, -sinT], axis=0)
    cc = np.tile(cc1, (1, B)).astype(bf)                # [128, R]
    ss = np.tile(ss1, (1, B)).astype(bf)

    # additive causal bias for diagonal 128-blocks, applied on PE via
    # ident.T @ bm: bm[p, q] = -1000 where k-pos p > q-pos q (strictly
    # above the diagonal), 0 elsewhere; exp then underflows to 0 there
    bm = np.tril(np.full((128, 128), -1000.0, np.float32), -1).astype(bf)

    in_maps = []
    for c in range(n_cores):
        qcols = []
        for i in range(NQ):
            h = c * NQ + i
            qcols.append(wq[:, h * HD:(h + 1) * HD][:, _EO_PERM])
        kcol = wk[:, c * HD:(c + 1) * HD][:, _EO_PERM]
        vcol = wv[:, c * HD:(c + 1) * HD]
        wqkv_c = np.concatenate(qcols + [kcol, vcol], axis=1)
        # [128, NM, KT, 128]: w[p, m, kt, j] = wqkv[kt*128+p, m*128+j]
        wqkv_c = np.ascontiguousarray(
            wqkv_c.reshape(KT, 128, 6, 128).transpose(1, 2, 0, 3)).astype(bf)
        wo_c = wo[c * NQ * HD:(c + 1) * NQ * HD, :]
        # [128, NQ, DIM]: wo[p, h, n] = wo_c[h*128+p, n]
        wo_c = np.ascontiguousarray(
            wo_c.reshape(NQ, 128, DIM).transpose(1, 0, 2)).astype(bf)
        in_maps.append({
            "xb": xb, "wqkv": wqkv_c, "wo": wo_c,
            "cc": cc, "ss": ss, "bm": bm,
        })
    return in_maps


_NC_CACHE = {}


def _get_nc(cfg: Cfg):
    if cfg not in _NC_CACHE:
        _NC_CACHE[cfg] = build_nc(cfg)
    return _NC_CACHE[cfg]


def kernel(x, wq, wk, wv, wo, freqs_cos, freqs_sin, mask, start_pos=0,
           **_ignored):
    from concourse.bass_utils import run_bass_kernel_spmd

    cfg = Cfg()
    nc = _get_nc(cfg)
    in_maps = shard_inputs(cfg, x, wq, wk, wv, wo, freqs_cos, freqs_sin, mask,
                           n_cores=8)
    res = run_bass_kernel_spmd(nc, in_maps, core_ids=list(range(8)))
    acc = np.zeros((cfg.R, cfg.DIM), np.float32)
    for c in range(8):
        acc += res.results[c]["out"].astype(np.float32)
    return acc.reshape(cfg.B, cfg.S, cfg.DIM)
